# revision 4
# baseline (speedup 1.0000x reference)
"""Trainium2 Bass kernel for nn_BaltNet (2-layer ConvLSTM + decoder + MLP head).

Sharding: data-parallel over batch B=8 (one sample per NeuronCore) for the
recurrent conv part; FC1's [131072, 256] contraction is K-sharded 8 ways
(AllToAll of the decoder features, per-core partial matmul, ReduceScatter).

Layout (v3): L1 shares A's h0 ky-groups, so h0 needs no separate Ba/Bb
placements (v1 kept 7 shifted-copy DMAs per quarter; the sync queue was
~10us backed up at every step boundary and re-throttled the PE).

  A  [105, 66, 68]: h0 ky=-1 @0-31, ky=0 @32-63, ky=+1 @64-95;
                    x  ky=-1 @96-98, ky=0 @99-101, ky=+1 @102-104
  H1 [ 96, 66, 68]: h1 ky=-1 @0-31, ky=0 @32-63, ky=+1 @64-95

  L0 / decoder: 3 kx passes of K=105 over A.
  L1:           3 kx passes of K=96 over A[0:96] + 3 of K=96 over H1.

All ky groups are 32-aligned: the h producer (DVE mul) writes the ky=0 slot
directly and only 2 shifted copies per quarter remain (sync + gpsimd DMA).

Pointwise: gates z = [i f o g] on 128 partitions; g-gate weights/bias
pre-scaled x2 so tanh(g) = 2*sigmoid(2g) - 1.  Cell state uses ping-pong
[64, 2, 4096] tiles (lane 0 = L0, lane 1 = L1/decoder) R/W per step:
R = [tg | c_old], the pair-mul writes W[0:64] = [i*tg | f*c], and a SWDGE
accumulate-DMA does the cross-partition add W[32:64] += W[0:32] = c_new
(saves 2 DVE ops/quarter vs v1's copy+add; DVE was 78%-busy).

Scheduling: L1 runs one step behind L0 and is emitted AFTER L0(t), so the
PE alternates L0(t) / L1(t-1) matmul bursts.  L0(t)'s tail pointwise
(tanh, h-mul, shifts) is DEFERRED and rides as a guest inside L1(t-1)'s
quarters (quarter q after L1's matmuls q+1 -- the one boundary row stays
WAR by program order), paired with L1's own tail so ONE Tanh op covers
both layers' quarter.  h0(t) placement completes under L1's burst and
L0(t+1) starts with no stall.

Tail: single AllToAll + single ReduceScatter (two phased A2As measured
5x slower than one -- per-collective overhead dominates at these sizes).
"""

import os
import sys

for _p in ("/opt/trn_rl_repo",):
    if _p not in sys.path and os.path.isdir(_p):
        sys.path.insert(0, _p)

import numpy as np

import concourse.bass as bass
import concourse.mybir as mybir
import concourse.tile as tile
from concourse import bacc
from concourse.bass_utils import run_bass_kernel_spmd

F16 = mybir.dt.float16
F32 = mybir.dt.float32
AF = mybir.ActivationFunctionType
OP = mybir.AluOpType

B, T, C, HID, H, W = 8, 24, 3, 32, 64, 64
G4 = 4 * HID            # 128 gate channels
PH, PW = H + 2, W + 4   # padded spatial: rows 0..65, interior cols 2..65
NPIX = H * W            # 4096
KSL = HID * NPIX // 8   # 16384 per-core FC1 K-slice
N_CORES = 8

TRACE = False           # test.py flips this for profiled runs
_CACHE = {}

KXS = (-1, 0, 1)


def _build_nc():
    nc = bacc.Bacc("TRN2", target_bir_lowering=False, debug=False,
                   num_devices=N_CORES)

    # ---- I/O -------------------------------------------------------------
    xp_d = nc.dram_tensor("xp", [T, C, PH, PW], F16, kind="ExternalInput")
    w0_d = nc.dram_tensor("w0", [105, 3 * G4], F16, kind="ExternalInput")
    w1h0_d = nc.dram_tensor("w1h0", [96, 3 * G4], F16, kind="ExternalInput")
    w1h1_d = nc.dram_tensor("w1h1", [96, 3 * G4], F16, kind="ExternalInput")
    wd_d = nc.dram_tensor("wd", [105, 3 * G4], F16, kind="ExternalInput")
    b0_d = nc.dram_tensor("b0", [G4, 1], F32, kind="ExternalInput")
    b1_d = nc.dram_tensor("b1", [G4, 1], F32, kind="ExternalInput")
    bd_d = nc.dram_tensor("bd", [G4, 1], F32, kind="ExternalInput")
    fw_d = nc.dram_tensor("fw", [128, 128 * 256], F16, kind="ExternalInput")
    fb_d = nc.dram_tensor("fb", [128, 2], F32, kind="ExternalInput")
    w2_d = nc.dram_tensor("w2", [128, 2 * 97], F16, kind="ExternalInput")
    b2_d = nc.dram_tensor("b2", [97, 1], F32, kind="ExternalInput")
    out_d = nc.dram_tensor("out", [97, 1], F32, kind="ExternalOutput")

    with tile.TileContext(nc) as tc:
        with (
            tc.tile_pool(name="state", bufs=1) as state,
            tc.tile_pool(name="const", bufs=1) as const,
            tc.tile_pool(name="sgate", bufs=3) as sgate,
            tc.tile_pool(name="scr", bufs=3) as scr,
            tc.tile_pool(name="psum", bufs=4, space="PSUM") as psum,
            tc.tile_pool(name="dram", bufs=1, space="DRAM") as dram,
        ):
            # ---- persistent SBUF state ----------------------------------
            A = state.tile([105, PH, PW], F16)
            H1t = state.tile([96, PH, PW], F16)
            # ping-pong cell state, lane 0 = L0, lane 1 = L1/decoder
            cst = [state.tile([64, 2, NPIX], F16, name="cst_a"),
                   state.tile([64, 2, NPIX], F16, name="cst_b")]
            hdc = state.tile([HID, NPIX], F16)    # decoder h (feat)

            # ---- constants ----------------------------------------------
            w0 = const.tile([105, 3 * G4], F16)
            w1h0 = const.tile([96, 3 * G4], F16)
            w1h1 = const.tile([96, 3 * G4], F16)
            wd = const.tile([105, 3 * G4], F16)
            b0 = const.tile([G4, 1], F32)
            b1 = const.tile([G4, 1], F32)
            bd = const.tile([G4, 1], F32)
            fw = const.tile([128, 128 * 256], F16)
            fb = const.tile([128, 2], F32)
            w2 = const.tile([128, 2 * 97], F16)
            b2 = const.tile([97, 1], F32)
            ft = const.tile([128, 8, 128], F16)   # A2A result, FC1 lhsT

            # zero-init on DVE (v1's Pool memsets delayed the first const
            # loads ~20us into the recurrence)
            nc.vector.memset(A[:], 0.0)
            nc.sync.dma_start(out=w0[:], in_=w0_d[:])
            nc.sync.dma_start(out=b0[:], in_=b0_d[:])
            nc.vector.memset(H1t[:], 0.0)
            nc.vector.memset(cst[0][:], 0.0)
            nc.vector.memset(cst[1][:], 0.0)
            CONST_LOADS = ((wd, wd_d), (bd, bd_d),
                           (fb, fb_d), (w2, w2_d), (b2, b2_d))

            # ---- DRAM bounce buffers for collectives --------------------
            a2a_in = dram.tile([HID, NPIX], F16)
            a2a_out = dram.tile([8, 128, 128], F16)
            z1part = dram.tile([8, 256], F32)
            z1red = dram.tile([256], F32)

            def xload(t):
                # x_t into A's 3 ky-groups (grp_ky[d] = x[d+ky])
                nc.sync.dma_start(out=A[99:102, :, :], in_=xp_d[t])
                nc.sync.dma_start(out=A[96:99, 1:PH, :],
                                  in_=xp_d[t, :, 0:PH - 1, :])
                nc.sync.dma_start(out=A[102:105, 0:PH - 1, :],
                                  in_=xp_d[t, :, 1:PH, :])

            def conv_step(srcs, bias, R, Wt, li, hdst,
                          guest=None, defer=False, immediate=False,
                          post_late=None):
                """One ConvLSTM cell step.  srcs: list of (buf, K, wt).
                R/Wt: ping-pong [64, 2, NPIX] cst tiles; li: lane (0=L0).

                Per quarter rt: matmul passes into a [128,1024] PSUM tile,
                then sigmoid + the c-chain: ts (tg into R), pair-mul
                ([i*tg|f*c] into W), SWDGE accumulate W[32:64]+=W[0:32].

                The tail (tanh, h-mul, shifted copies) is emitted per the
                policy: defer=True returns `rest` closures for a later
                host conv; guest= emits a prior conv's rest quarters after
                this conv's matmuls (q after MMs q+1), sharing ONE Tanh op
                across both layers' lanes; immediate=True (decoder) emits
                the tail right after each quarter's c-update.
                """
                npass = len(srcs) * 3
                S = sgate.tile([G4, NPIX], F16, tag="S")

                def rest(q, tht, ci):
                    s_ = slice(q * 1024, (q + 1) * 1024)
                    r0 = 16 * q
                    th = tht[64:96, ci, :]
                    if hdst is hdc:
                        nc.vector.tensor_mul(hdc[:, s_], S[64:96, s_], th)
                    else:
                        nc.vector.tensor_mul(
                            hdst[32:64, r0 + 1:r0 + 17, 2:66],
                            S[64:96, s_], th)
                        nc.sync.dma_start(
                            out=hdst[0:32, r0 + 2:r0 + 18, :],
                            in_=hdst[32:64, r0 + 1:r0 + 17, :])
                        nc.gpsimd.dma_start(
                            out=hdst[64:96, r0:r0 + 16, :],
                            in_=hdst[32:64, r0 + 1:r0 + 17, :])
                    if post_late is not None:
                        post_late(q)

                def own_tail(q):
                    # unpaired: tanh on this lane only
                    tht = scr.tile([96, 2, 1024], F16, tag="tht")
                    nc.scalar.activation(
                        out=tht[64:96, li:li + 1, :],
                        in_=Wt[32:64, li:li + 1, q * 1024:(q + 1) * 1024],
                        func=AF.Tanh)
                    rest(q, tht, li)

                def paired_tail(q):
                    # one tanh covers guest lane 0 + own lane 1
                    tht = scr.tile([96, 2, 1024], F16, tag="tht")
                    nc.scalar.activation(
                        out=tht[64:96, :, :],
                        in_=Wt[32:64, :, q * 1024:(q + 1) * 1024],
                        func=AF.Tanh)
                    guest(q, tht, 0)
                    rest(q, tht, 1)

                tail = paired_tail if guest is not None else own_tail

                for rt in range(4):
                    s_ = slice(rt * 1024, (rt + 1) * 1024)
                    csl = slice(rt * 1024, (rt + 1) * 1024)
                    pz = psum.tile([G4, 1024], F32, tag="z", name=f"pz{rt}")
                    ip = 0
                    for buf, K, wt in srcs:
                        for kxi, kx in enumerate(KXS):
                            lhs = wt[:, kxi * G4:(kxi + 1) * G4]
                            for hh in range(2):
                                r0 = 16 * rt + 8 * hh
                                rhs = buf[0:K, r0 + 1:r0 + 9, 2 + kx:66 + kx]
                                nc.tensor.matmul(
                                    pz[:, 512 * hh:512 * hh + 512],
                                    lhs, rhs, start=(ip == 0),
                                    stop=(ip == npass - 1))
                            ip += 1
                    if not defer and not immediate and rt >= 1:
                        tail(rt - 1)
                    nc.scalar.activation(out=S[:, s_], in_=pz[:],
                                         func=AF.Sigmoid,
                                         bias=bias[:, 0:1], scale=1.0)
                    # tg = 2*sigmoid(2g) - 1 into R's (dead) lower half
                    nc.vector.tensor_scalar(
                        out=R[0:32, li, csl], in0=S[96:128, s_],
                        scalar1=2.0, scalar2=-1.0, op0=OP.mult, op1=OP.add)
                    # [i*tg | f*c]
                    nc.vector.tensor_mul(Wt[0:64, li, csl], S[0:64, s_],
                                         R[0:64, li, csl])
                    # c_new = i*tg + f*c (cross-partition add via CCE)
                    nc.gpsimd.dma_start(out=Wt[32:64, li, csl],
                                        in_=Wt[0:32, li, csl],
                                        accum_op=OP.add)
                    if immediate:
                        tail(rt)
                if not defer and not immediate:
                    tail(3)
                return rest

            # ================= recurrent steps ===========================
            # L1 one step behind L0, emitted after it; L0(t)'s tail rides
            # as guest in L1(t-1) so h0(t) placement hides under L1's MMs.
            # Ping-pong parity: L0(t): R=cst[t%2], W=cst[(t+1)%2];
            # L1(s):  R=cst[(s+1)%2], W=cst[s%2]  -> same W tile as L0(t)
            # for s = t-1, which the shared-lane paired tanh requires.
            xload(0)
            pend = None
            for t in range(T):
                pend = conv_step([(A, 105, w0)], b0,
                                 cst[t % 2], cst[(t + 1) % 2], 0, A,
                                 defer=(t > 0))
                if t + 1 < T:
                    xload(t + 1)
                if t > 0:
                    s = t - 1
                    conv_step([(A, 96, w1h0), (H1t, 96, w1h1)], b1,
                              cst[(s + 1) % 2], cst[s % 2], 1, H1t,
                              guest=pend)
                if t == 0:
                    nc.gpsimd.dma_start(out=w1h0[:], in_=w1h0_d[:])
                    nc.gpsimd.dma_start(out=w1h1[:], in_=w1h1_d[:])
                    nc.gpsimd.dma_start(out=b1[:], in_=b1_d[:])
                elif 2 <= t < 2 + len(CONST_LOADS):
                    dst, src = CONST_LOADS[t - 2]
                    nc.gpsimd.dma_start(out=dst[:], in_=src[:])
                if 2 <= t < 2 + 16:
                    # trickle in the 8.4MB fc1 weight (tail-only)
                    i = t - 2
                    nc.gpsimd.dma_start(out=fw[:, i * 2048:(i + 1) * 2048],
                                        in_=fw_d[:, i * 2048:(i + 1) * 2048])
            # final L1 step (s = T-1): hT fans out into A's h slots; its
            # tail must be inline (the decoder READS those writes).
            s = T - 1
            conv_step([(A, 96, w1h0), (H1t, 96, w1h1)], b1,
                      cst[(s + 1) % 2], cst[s % 2], 1, A)

            # ================= decoder step ==============================
            def feed(q):
                # stream each decoder quarter into the A2A input
                sl = slice(q * 1024, (q + 1) * 1024)
                nc.sync.dma_start(out=a2a_in[:, sl], in_=hdc[:, sl])

            conv_step([(A, 105, wd)], bd,
                      cst[(T + 1) % 2], cst[T % 2], 1, hdc,
                      immediate=True, post_late=feed)

            # ================= FC head ===================================
            nc.gpsimd.collective_compute(
                "AllToAll", OP.bypass,
                replica_groups=[list(range(N_CORES))],
                ins=[a2a_in[:].opt()], outs=[a2a_out[:].opt()])
            # transposed load with K-index q = p*128 + k2
            nc.sync.dma_start(
                out=ft[:],
                in_=a2a_out[:].rearrange("m p k -> p m k"))

            psz = psum.tile([8, 256], F32, tag="z")
            for k2 in range(128):
                nc.tensor.matmul(psz[:], ft[:, :, k2],
                                 fw[:, k2 * 256:(k2 + 1) * 256],
                                 start=(k2 == 0), stop=(k2 == 127))
            z1s = scr.tile([8, 256], F32, tag="z1")
            nc.vector.tensor_copy(z1s[:], psz[:])
            nc.sync.dma_start(out=z1part[:], in_=z1s[:])
            nc.gpsimd.collective_compute(
                "ReduceScatter", OP.add,
                replica_groups=[list(range(N_CORES))],
                ins=[z1part[:].opt()], outs=[z1red[:].opt()])

            zr = scr.tile([128, 2], F32, tag="zr")
            nc.sync.dma_start(out=zr[:],
                              in_=z1red[:].rearrange("(j p) -> p j", p=128))
            zrb = scr.tile([128, 2], F32, tag="zrb")
            nc.vector.tensor_add(zrb[:], zr[:], fb[:])
            h256 = scr.tile([128, 2], F16, tag="h256")
            nc.vector.tensor_scalar_max(h256[:], zrb[:], 0.0)

            ps2 = psum.tile([97, 1], F32, tag="z")
            for j in range(2):
                nc.tensor.matmul(ps2[:], w2[:, j * 97:(j + 1) * 97],
                                 h256[:, j:j + 1],
                                 start=(j == 0), stop=(j == 1))
            outs = scr.tile([97, 1], F32, tag="outs")
            nc.vector.tensor_add(outs[:], ps2[:], b2[:])
            nc.sync.dma_start(out=out_d[:], in_=outs[:])

    nc.compile()
    return nc


def _prep_inputs(x, Wenc0, benc0, Wenc1, benc1, Wdec, bdec,
                 fc1_w, fc1_b, fc2_w, fc2_b):
    """Host-side: pad/reorder/cast everything into device layouts."""
    f16 = np.float16

    def conv_w_l0(Wk):
        # Wk [128, 35, 3, 3]; ref channel order [x(3), h(32)]
        Wk = np.asarray(Wk, np.float32).copy()
        Wk[96:128] *= 2.0
        out = np.zeros((105, 3 * G4), np.float32)
        for kxi in range(3):
            for dy in range(3):
                out[32 * dy:32 * dy + 32, kxi * G4:(kxi + 1) * G4] = \
                    Wk[:, 3:, dy, kxi].T
                out[96 + 3 * dy:99 + 3 * dy, kxi * G4:(kxi + 1) * G4] = \
                    Wk[:, :3, dy, kxi].T
        return out.astype(f16)

    def conv_w_l1(Wk):
        # Wk [128, 64, 3, 3]; ref channel order [h0(32), h1(32)]
        Wk = np.asarray(Wk, np.float32).copy()
        Wk[96:128] *= 2.0
        wh0 = np.zeros((96, 3 * G4), np.float32)
        wh1 = np.zeros((96, 3 * G4), np.float32)
        for kxi in range(3):
            for dy in range(3):
                wh0[32 * dy:32 * dy + 32, kxi * G4:(kxi + 1) * G4] = \
                    Wk[:, 0:32, dy, kxi].T
                wh1[32 * dy:32 * dy + 32, kxi * G4:(kxi + 1) * G4] = \
                    Wk[:, 32:64, dy, kxi].T
        return wh0.astype(f16), wh1.astype(f16)

    def bias_v(b):
        b = np.asarray(b, np.float32).copy()
        b[96:128] *= 2.0
        return b.reshape(G4, 1)

    w0_full = conv_w_l0(Wenc0)
    wd_full = conv_w_l0(Wdec)
    w1h0, w1h1 = conv_w_l1(Wenc1)

    xpad = np.zeros((B, T, C, PH, PW), f16)
    xpad[:, :, :, 1:65, 2:66] = np.asarray(x, np.float32)

    fc1_w = np.asarray(fc1_w, np.float32)
    fb = np.asarray(fc1_b, np.float32).reshape(2, 128).T.copy()  # [128, 2]
    w2 = np.asarray(fc2_w, np.float32).T.reshape(2, 128, 97)
    w2 = np.ascontiguousarray(w2.transpose(1, 0, 2)).reshape(128, 2 * 97)
    b2 = np.asarray(fc2_b, np.float32).reshape(97, 1)

    in_maps = []
    for k in range(N_CORES):
        w1k = fc1_w[:, k * KSL:(k + 1) * KSL].T            # [16384, 256]
        # K-index q = p*128 + k2  ->  fw[p, k2*256+n] = w1k[p*128 + k2, n]
        fwk = w1k.reshape(128, 128 * 256)
        in_maps.append({
            "xp": np.ascontiguousarray(xpad[k]),
            "w0": w0_full, "w1h0": w1h0, "w1h1": w1h1, "wd": wd_full,
            "b0": bias_v(benc0), "b1": bias_v(benc1), "bd": bias_v(bdec),
            "fw": fwk.astype(f16), "fb": fb,
            "w2": w2.astype(f16), "b2": b2,
        })
    return in_maps


def kernel(**inputs):
    if "nc" not in _CACHE:
        _CACHE["nc"] = _build_nc()
    nc = _CACHE["nc"]
    in_maps = _prep_inputs(**inputs)
    res = run_bass_kernel_spmd(nc, in_maps, core_ids=list(range(N_CORES)),
                               trace=TRACE)
    _CACHE["last_result"] = res
    out = np.stack([res.results[k]["out"][:, 0] for k in range(N_CORES)])
    return out.astype(np.float32)


# revision 5
# speedup vs baseline: 1.1709x; 1.1709x over previous
"""Trainium2 Bass kernel for nn_BaltNet (2-layer ConvLSTM + decoder + MLP head).

Sharding: data-parallel over batch B=8 (one sample per NeuronCore) for the
recurrent conv part; FC1's [131072, 256] contraction is K-sharded 8 ways
(AllToAll of the decoder features, per-core partial matmul, ReduceScatter).

Layout (v4): L1 shares A's h0 ky-groups, so h0 needs no separate Ba/Bb
placements (v1 kept 7 shifted-copy DMAs per quarter; the sync queue was
~10us backed up at every step boundary and re-throttled the PE).

  A  [105, 66, 68]: h0 ky=-1 @0-31, ky=0 @32-63, ky=+1 @64-95;
                    x  ky=-1 @96-98, ky=0 @99-101, ky=+1 @102-104
  H1 [ 96, 66, 68]: h1 ky=-1 @0-31, ky=0 @32-63, ky=+1 @64-95

  L0 / decoder: 3 kx passes of K=105 over A.
  L1:           3 kx passes of K=96 over A[0:96] + 3 of K=96 over H1.

All ky groups are 32-aligned: the h producer (DVE mul) writes the ky=0 slot
directly and only 2 shifted copies per quarter remain (sync + gpsimd DMA).

Pointwise: gates z = [i f o g] on 128 partitions; g-gate weights/bias
pre-scaled x2 so tanh(g) = 2*sigmoid(2g) - 1.  Cell state uses ping-pong
[64, 2, 4096] tiles (lane 0 = L0, lane 1 = L1/decoder) R/W per step:
R = [tg | c_old], the pair-mul writes W[0:64] = [i*tg | f*c], and a SWDGE
accumulate-DMA does the cross-partition add W[32:64] += W[0:32] = c_new
(saves 2 DVE ops/quarter vs v1's copy+add; DVE was 78%-busy).  The decoder
and final-L1 steps use a DVE copy+add instead: at the tail nothing hides
the ~3us SWDGE flight, and the shorter chain triggers the AllToAll sooner.

Scheduling: L1 one step behind L0, emitted after it, so the PE alternates
L0(t) / L1(t-1) bursts.  L0(t) computes its tanh inline (gated only by its
own c-update), and ONLY its h-placements (h-mul + 2 shifted copies) are
deferred to ride in L1(t-1): quarter q's h-mul + ky+1 copy after L1's
matmuls q (their A-rows WAR-clear there), the ky-1 copy after matmuls q+1
(one boundary row).  Placements finish under L1's burst, so L0(t+1) starts
stall-free and the PE's HAM activity throttle stays released (in v1-v3 a
per-step PE gap re-throttled the clock to 1.2GHz for ~half of every step).

Tail: single AllToAll + single ReduceScatter (two phased A2As measured 5x
slower than one -- per-collective overhead dominates at these sizes).
"""

import os
import sys

for _p in ("/opt/trn_rl_repo",):
    if _p not in sys.path and os.path.isdir(_p):
        sys.path.insert(0, _p)

import numpy as np

import concourse.bass as bass
import concourse.mybir as mybir
import concourse.tile as tile
from concourse import bacc
from concourse.bass_utils import run_bass_kernel_spmd

F16 = mybir.dt.float16
F32 = mybir.dt.float32
AF = mybir.ActivationFunctionType
OP = mybir.AluOpType

B, T, C, HID, H, W = 8, 24, 3, 32, 64, 64
G4 = 4 * HID            # 128 gate channels
PH, PW = H + 2, W + 4   # padded spatial: rows 0..65, interior cols 2..65
NPIX = H * W            # 4096
KSL = HID * NPIX // 8   # 16384 per-core FC1 K-slice
N_CORES = 8

TRACE = False           # test.py flips this for profiled runs
_CACHE = {}

KXS = (-1, 0, 1)


def _build_nc():
    nc = bacc.Bacc("TRN2", target_bir_lowering=False, debug=False,
                   num_devices=N_CORES)

    # ---- I/O -------------------------------------------------------------
    xp_d = nc.dram_tensor("xp", [T, C, PH, PW], F16, kind="ExternalInput")
    w0_d = nc.dram_tensor("w0", [105, 3 * G4], F16, kind="ExternalInput")
    w1h0_d = nc.dram_tensor("w1h0", [96, 3 * G4], F16, kind="ExternalInput")
    w1h1_d = nc.dram_tensor("w1h1", [96, 3 * G4], F16, kind="ExternalInput")
    wd_d = nc.dram_tensor("wd", [105, 3 * G4], F16, kind="ExternalInput")
    b0_d = nc.dram_tensor("b0", [G4, 1], F32, kind="ExternalInput")
    b1_d = nc.dram_tensor("b1", [G4, 1], F32, kind="ExternalInput")
    bd_d = nc.dram_tensor("bd", [G4, 1], F32, kind="ExternalInput")
    fw_d = nc.dram_tensor("fw", [128, 128 * 256], F16, kind="ExternalInput")
    fb_d = nc.dram_tensor("fb", [128, 2], F32, kind="ExternalInput")
    w2_d = nc.dram_tensor("w2", [128, 2 * 97], F16, kind="ExternalInput")
    b2_d = nc.dram_tensor("b2", [97, 1], F32, kind="ExternalInput")
    out_d = nc.dram_tensor("out", [97, 1], F32, kind="ExternalOutput")

    with tile.TileContext(nc) as tc:
        with (
            tc.tile_pool(name="state", bufs=1) as state,
            tc.tile_pool(name="const", bufs=1) as const,
            tc.tile_pool(name="sgate", bufs=3) as sgate,
            tc.tile_pool(name="scr", bufs=3) as scr,
            tc.tile_pool(name="psum", bufs=4, space="PSUM") as psum,
            tc.tile_pool(name="dram", bufs=1, space="DRAM") as dram,
        ):
            # ---- persistent SBUF state ----------------------------------
            A = state.tile([105, PH, PW], F16)
            H1t = state.tile([96, PH, PW], F16)
            # ping-pong cell state, lane 0 = L0, lane 1 = L1/decoder
            cst = [state.tile([64, 2, NPIX], F16, name="cst_a"),
                   state.tile([64, 2, NPIX], F16, name="cst_b")]
            hdc = state.tile([HID, NPIX], F16)    # decoder h (feat)

            # ---- constants ----------------------------------------------
            w0 = const.tile([105, 3 * G4], F16)
            w1h0 = const.tile([96, 3 * G4], F16)
            w1h1 = const.tile([96, 3 * G4], F16)
            wd = const.tile([105, 3 * G4], F16)
            b0 = const.tile([G4, 1], F32)
            b1 = const.tile([G4, 1], F32)
            bd = const.tile([G4, 1], F32)
            fw = const.tile([128, 128 * 256], F16)
            fb = const.tile([128, 2], F32)
            w2 = const.tile([128, 2 * 97], F16)
            b2 = const.tile([97, 1], F32)
            ft = const.tile([128, 8, 128], F16)   # A2A result, FC1 lhsT

            # zero-init on DVE; first matmuls only need A + w0 + b0 + x(0)
            nc.vector.memset(A[:], 0.0)
            nc.sync.dma_start(out=w0[:], in_=w0_d[:])
            nc.sync.dma_start(out=b0[:], in_=b0_d[:])
            nc.vector.memset(H1t[:], 0.0)
            nc.vector.memset(cst[0][:], 0.0)
            nc.vector.memset(cst[1][:], 0.0)

            # ---- DRAM bounce buffers for collectives --------------------
            a2a_in = dram.tile([HID, NPIX], F16)
            a2a_out = dram.tile([8, 128, 128], F16)
            z1part = dram.tile([8, 256], F32)
            z1red = dram.tile([256], F32)

            def xload(t):
                # x_t into A's 3 ky-groups (grp_ky[d] = x[d+ky])
                nc.sync.dma_start(out=A[99:102, :, :], in_=xp_d[t])
                nc.sync.dma_start(out=A[96:99, 1:PH, :],
                                  in_=xp_d[t, :, 0:PH - 1, :])
                nc.sync.dma_start(out=A[102:105, 0:PH - 1, :],
                                  in_=xp_d[t, :, 1:PH, :])

            def conv_step(srcs, bias, R, Wt, li, hdst,
                          defer=False, guest=None, immediate=False,
                          dve_add=False, post_late=None):
                """One ConvLSTM cell step.  srcs: list of (buf, K, wt).
                R/Wt: ping-pong [64, 2, NPIX] cst tiles; li: lane (0=L0).

                Per quarter rt: matmul passes into a [128,1024] PSUM tile,
                then sigmoid + the c-chain: ts (tg into R), pair-mul
                ([i*tg|f*c] into W), cross-partition add W[32:64]+=W[0:32]
                (SWDGE accumulate, or DVE copy+add when dve_add).

                Tail policy: defer=True computes tanh inline but returns
                (g_a, g_b) closures -- g_a(q) = h-mul + ky+1 copy, g_b(q) =
                ky-1 copy -- for the next conv to host; guest=(g_a, g_b)
                emits them after this conv's matmuls q / q+1; immediate=True
                (decoder) emits the full tail right after each quarter.
                """
                npass = len(srcs) * 3
                S = sgate.tile([G4, NPIX], F16, tag="S")
                thts = {}

                def tanh_q(q):
                    tht = scr.tile([96, 1024], F16, tag="tht")
                    nc.scalar.activation(
                        out=tht[64:96, :],
                        in_=Wt[32:64, li, q * 1024:(q + 1) * 1024],
                        func=AF.Tanh)
                    thts[q] = tht

                def hmul_kyp1(q):
                    if q not in thts:
                        tanh_q(q)
                    s_ = slice(q * 1024, (q + 1) * 1024)
                    r0 = 16 * q
                    th = thts[q][64:96, :]
                    if hdst is hdc:
                        nc.vector.tensor_mul(hdc[:, s_], S[64:96, s_], th)
                    else:
                        nc.vector.tensor_mul(
                            hdst[32:64, r0 + 1:r0 + 17, 2:66],
                            S[64:96, s_], th)
                        nc.gpsimd.dma_start(
                            out=hdst[64:96, r0:r0 + 16, :],
                            in_=hdst[32:64, r0 + 1:r0 + 17, :])
                    if post_late is not None:
                        post_late(q)

                def kym1(q):
                    if hdst is hdc:
                        return
                    r0 = 16 * q
                    nc.sync.dma_start(
                        out=hdst[0:32, r0 + 2:r0 + 18, :],
                        in_=hdst[32:64, r0 + 1:r0 + 17, :])

                for rt in range(4):
                    s_ = slice(rt * 1024, (rt + 1) * 1024)
                    csl = slice(rt * 1024, (rt + 1) * 1024)
                    pz = psum.tile([G4, 1024], F32, tag="z", name=f"pz{rt}")
                    ip = 0
                    for buf, K, wt in srcs:
                        for kxi, kx in enumerate(KXS):
                            lhs = wt[:, kxi * G4:(kxi + 1) * G4]
                            for hh in range(2):
                                r0 = 16 * rt + 8 * hh
                                rhs = buf[0:K, r0 + 1:r0 + 9, 2 + kx:66 + kx]
                                nc.tensor.matmul(
                                    pz[:, 512 * hh:512 * hh + 512],
                                    lhs, rhs, start=(ip == 0),
                                    stop=(ip == npass - 1))
                            ip += 1
                    if guest is not None:
                        guest[0](rt)               # prev conv h-mul + ky+1
                        if rt >= 1:
                            guest[1](rt - 1)       # prev conv ky-1 copy
                    if not defer and not immediate and rt >= 1:
                        q = rt - 1
                        tanh_q(q)
                        hmul_kyp1(q)
                        kym1(q)
                    nc.scalar.activation(out=S[:, s_], in_=pz[:],
                                         func=AF.Sigmoid,
                                         bias=bias[:, 0:1], scale=1.0)
                    # tg = 2*sigmoid(2g) - 1 into R's (dead) lower half
                    nc.vector.tensor_scalar(
                        out=R[0:32, li, csl], in0=S[96:128, s_],
                        scalar1=2.0, scalar2=-1.0, op0=OP.mult, op1=OP.add)
                    # [i*tg | f*c]
                    nc.vector.tensor_mul(Wt[0:64, li, csl], S[0:64, s_],
                                         R[0:64, li, csl])
                    # c_new = i*tg + f*c (cross-partition add)
                    if dve_add:
                        u1c = scr.tile([32, 1024], F16, tag="u1c")
                        nc.vector.tensor_copy(u1c[:], Wt[32:64, li, csl])
                        nc.vector.tensor_add(Wt[32:64, li, csl],
                                             Wt[0:32, li, csl], u1c[:])
                    else:
                        nc.gpsimd.dma_start(out=Wt[32:64, li, csl],
                                            in_=Wt[0:32, li, csl],
                                            accum_op=OP.add)
                    if defer and rt >= 1:
                        tanh_q(rt - 1)             # inline; placement rides
                    if immediate:
                        tanh_q(rt)
                        hmul_kyp1(rt)
                        kym1(rt)
                if guest is not None:
                    guest[1](3)
                if not defer and not immediate:
                    tanh_q(3)
                    hmul_kyp1(3)
                    kym1(3)
                return (hmul_kyp1, kym1)

            # ================= recurrent steps ===========================
            # Ping-pong parity: L0(t): R=cst[t%2], W=cst[(t+1)%2];
            # L1(s): R=cst[(s+1)%2], W=cst[s%2].
            xload(0)
            # remaining consts; behind w0/b0/x(0) on sync so the first
            # matmuls start ~2.5us in
            for dst, src in ((w1h0, w1h0_d), (w1h1, w1h1_d), (b1, b1_d),
                             (wd, wd_d), (bd, bd_d), (fb, fb_d),
                             (w2, w2_d), (b2, b2_d)):
                nc.sync.dma_start(out=dst[:], in_=src[:])
            pend = None
            for t in range(T):
                pend = conv_step([(A, 105, w0)], b0,
                                 cst[t % 2], cst[(t + 1) % 2], 0, A,
                                 defer=(t > 0))
                if t + 1 < T:
                    xload(t + 1)
                if t > 0:
                    s = t - 1
                    conv_step([(A, 96, w1h0), (H1t, 96, w1h1)], b1,
                              cst[(s + 1) % 2], cst[s % 2], 1, H1t,
                              guest=pend)
                if 1 <= t < 1 + 16:
                    # trickle in the 8.4MB fc1 weight (tail-only)
                    i = t - 1
                    nc.gpsimd.dma_start(out=fw[:, i * 2048:(i + 1) * 2048],
                                        in_=fw_d[:, i * 2048:(i + 1) * 2048])
            # final L1 step (s = T-1): hT fans out into A's h slots; its
            # tail must be inline (the decoder READS those writes).
            s = T - 1
            conv_step([(A, 96, w1h0), (H1t, 96, w1h1)], b1,
                      cst[(s + 1) % 2], cst[s % 2], 1, A, dve_add=True)

            # ================= decoder step ==============================
            def feed(q):
                # stream each decoder quarter into the A2A input
                sl = slice(q * 1024, (q + 1) * 1024)
                nc.sync.dma_start(out=a2a_in[:, sl], in_=hdc[:, sl])

            conv_step([(A, 105, wd)], bd,
                      cst[(T + 1) % 2], cst[T % 2], 1, hdc,
                      immediate=True, dve_add=True, post_late=feed)

            # ================= FC head ===================================
            nc.gpsimd.collective_compute(
                "AllToAll", OP.bypass,
                replica_groups=[list(range(N_CORES))],
                ins=[a2a_in[:].opt()], outs=[a2a_out[:].opt()])
            # transposed load with K-index q = p*128 + k2
            nc.sync.dma_start(
                out=ft[:],
                in_=a2a_out[:].rearrange("m p k -> p m k"))

            psz = psum.tile([8, 256], F32, tag="z")
            for k2 in range(128):
                nc.tensor.matmul(psz[:], ft[:, :, k2],
                                 fw[:, k2 * 256:(k2 + 1) * 256],
                                 start=(k2 == 0), stop=(k2 == 127))
            z1s = scr.tile([8, 256], F32, tag="z1")
            nc.vector.tensor_copy(z1s[:], psz[:])
            nc.sync.dma_start(out=z1part[:], in_=z1s[:])
            nc.gpsimd.collective_compute(
                "ReduceScatter", OP.add,
                replica_groups=[list(range(N_CORES))],
                ins=[z1part[:].opt()], outs=[z1red[:].opt()])

            zr = scr.tile([128, 2], F32, tag="zr")
            nc.sync.dma_start(out=zr[:],
                              in_=z1red[:].rearrange("(j p) -> p j", p=128))
            zrb = scr.tile([128, 2], F32, tag="zrb")
            nc.vector.tensor_add(zrb[:], zr[:], fb[:])
            h256 = scr.tile([128, 2], F16, tag="h256")
            nc.vector.tensor_scalar_max(h256[:], zrb[:], 0.0)

            ps2 = psum.tile([97, 1], F32, tag="z")
            for j in range(2):
                nc.tensor.matmul(ps2[:], w2[:, j * 97:(j + 1) * 97],
                                 h256[:, j:j + 1],
                                 start=(j == 0), stop=(j == 1))
            outs = scr.tile([97, 1], F32, tag="outs")
            nc.vector.tensor_add(outs[:], ps2[:], b2[:])
            nc.sync.dma_start(out=out_d[:], in_=outs[:])

    nc.compile()
    return nc


def _prep_inputs(x, Wenc0, benc0, Wenc1, benc1, Wdec, bdec,
                 fc1_w, fc1_b, fc2_w, fc2_b):
    """Host-side: pad/reorder/cast everything into device layouts."""
    f16 = np.float16

    def conv_w_l0(Wk):
        # Wk [128, 35, 3, 3]; ref channel order [x(3), h(32)]
        Wk = np.asarray(Wk, np.float32).copy()
        Wk[96:128] *= 2.0
        out = np.zeros((105, 3 * G4), np.float32)
        for kxi in range(3):
            for dy in range(3):
                out[32 * dy:32 * dy + 32, kxi * G4:(kxi + 1) * G4] = \
                    Wk[:, 3:, dy, kxi].T
                out[96 + 3 * dy:99 + 3 * dy, kxi * G4:(kxi + 1) * G4] = \
                    Wk[:, :3, dy, kxi].T
        return out.astype(f16)

    def conv_w_l1(Wk):
        # Wk [128, 64, 3, 3]; ref channel order [h0(32), h1(32)]
        Wk = np.asarray(Wk, np.float32).copy()
        Wk[96:128] *= 2.0
        wh0 = np.zeros((96, 3 * G4), np.float32)
        wh1 = np.zeros((96, 3 * G4), np.float32)
        for kxi in range(3):
            for dy in range(3):
                wh0[32 * dy:32 * dy + 32, kxi * G4:(kxi + 1) * G4] = \
                    Wk[:, 0:32, dy, kxi].T
                wh1[32 * dy:32 * dy + 32, kxi * G4:(kxi + 1) * G4] = \
                    Wk[:, 32:64, dy, kxi].T
        return wh0.astype(f16), wh1.astype(f16)

    def bias_v(b):
        b = np.asarray(b, np.float32).copy()
        b[96:128] *= 2.0
        return b.reshape(G4, 1)

    w0_full = conv_w_l0(Wenc0)
    wd_full = conv_w_l0(Wdec)
    w1h0, w1h1 = conv_w_l1(Wenc1)

    xpad = np.zeros((B, T, C, PH, PW), f16)
    xpad[:, :, :, 1:65, 2:66] = np.asarray(x, np.float32)

    fc1_w = np.asarray(fc1_w, np.float32)
    fb = np.asarray(fc1_b, np.float32).reshape(2, 128).T.copy()  # [128, 2]
    w2 = np.asarray(fc2_w, np.float32).T.reshape(2, 128, 97)
    w2 = np.ascontiguousarray(w2.transpose(1, 0, 2)).reshape(128, 2 * 97)
    b2 = np.asarray(fc2_b, np.float32).reshape(97, 1)

    in_maps = []
    for k in range(N_CORES):
        w1k = fc1_w[:, k * KSL:(k + 1) * KSL].T            # [16384, 256]
        # K-index q = p*128 + k2  ->  fw[p, k2*256+n] = w1k[p*128 + k2, n]
        fwk = w1k.reshape(128, 128 * 256)
        in_maps.append({
            "xp": np.ascontiguousarray(xpad[k]),
            "w0": w0_full, "w1h0": w1h0, "w1h1": w1h1, "wd": wd_full,
            "b0": bias_v(benc0), "b1": bias_v(benc1), "bd": bias_v(bdec),
            "fw": fwk.astype(f16), "fb": fb,
            "w2": w2.astype(f16), "b2": b2,
        })
    return in_maps


def kernel(**inputs):
    if "nc" not in _CACHE:
        _CACHE["nc"] = _build_nc()
    nc = _CACHE["nc"]
    in_maps = _prep_inputs(**inputs)
    res = run_bass_kernel_spmd(nc, in_maps, core_ids=list(range(N_CORES)),
                               trace=TRACE)
    _CACHE["last_result"] = res
    out = np.stack([res.results[k]["out"][:, 0] for k in range(N_CORES)])
    return out.astype(np.float32)


# revision 6
# speedup vs baseline: 1.2159x; 1.0384x over previous
"""Trainium2 Bass kernel for nn_BaltNet (2-layer ConvLSTM + decoder + MLP head).

Sharding: data-parallel over batch B=8 (one sample per NeuronCore) for the
recurrent conv part; FC1's [131072, 256] contraction is K-sharded 8 ways
(AllToAll of the decoder features, per-core partial matmul, ReduceScatter).

Layout (v4): L1 shares A's h0 ky-groups, so h0 needs no separate Ba/Bb
placements (v1 kept 7 shifted-copy DMAs per quarter; the sync queue was
~10us backed up at every step boundary and re-throttled the PE).

  A  [105, 66, 68]: h0 ky=-1 @0-31, ky=0 @32-63, ky=+1 @64-95;
                    x  ky=-1 @96-98, ky=0 @99-101, ky=+1 @102-104
  H1 [ 96, 66, 68]: h1 ky=-1 @0-31, ky=0 @32-63, ky=+1 @64-95

  L0 / decoder: 3 kx passes of K=105 over A.
  L1:           3 kx passes of K=96 over A[0:96] + 3 of K=96 over H1.

All ky groups are 32-aligned: the h producer (DVE mul) writes the ky=0 slot
directly and only 2 shifted copies per quarter remain (sync + gpsimd DMA).

Pointwise: gates z = [i f o g] on 128 partitions; g-gate weights/bias
pre-scaled x2 so tanh(g) = 2*sigmoid(2g) - 1.  Cell state uses ping-pong
[64, 2, 4096] tiles (lane 0 = L0, lane 1 = L1/decoder) R/W per step:
R = [tg | c_old], the pair-mul writes W[0:64] = [i*tg | f*c], and a SWDGE
accumulate-DMA does the cross-partition add W[32:64] += W[0:32] = c_new
(saves 2 DVE ops/quarter vs v1's copy+add; DVE was 78%-busy).  The decoder
and final-L1 steps use a DVE copy+add instead: at the tail nothing hides
the ~3us SWDGE flight, and the shorter chain triggers the AllToAll sooner.

Scheduling: L1 one step behind L0, emitted after it, so the PE alternates
L0(t) / L1(t-1) bursts.  L0(t) computes its tanh inline (gated only by its
own c-update), and ONLY its h-placements (h-mul + 2 shifted copies) are
deferred to ride in L1(t-1): quarter q's h-mul + ky+1 copy after L1's
matmuls q (their A-rows WAR-clear there), the ky-1 copy after matmuls q+1
(one boundary row).  Placements finish under L1's burst, so L0(t+1) starts
stall-free and the PE's HAM activity throttle stays released (in v1-v3 a
per-step PE gap re-throttled the clock to 1.2GHz for ~half of every step).

Tail: single AllToAll + single ReduceScatter (two phased A2As measured 5x
slower than one -- per-collective overhead dominates at these sizes).
"""

import os
import sys

for _p in ("/opt/trn_rl_repo",):
    if _p not in sys.path and os.path.isdir(_p):
        sys.path.insert(0, _p)

import numpy as np

import concourse.bass as bass
import concourse.mybir as mybir
import concourse.tile as tile
from concourse import bacc
from concourse.bass_utils import run_bass_kernel_spmd

F16 = mybir.dt.float16
F32 = mybir.dt.float32
AF = mybir.ActivationFunctionType
OP = mybir.AluOpType

B, T, C, HID, H, W = 8, 24, 3, 32, 64, 64
G4 = 4 * HID            # 128 gate channels
PH, PW = H + 2, W + 4   # padded spatial: rows 0..65, interior cols 2..65
NPIX = H * W            # 4096
KSL = HID * NPIX // 8   # 16384 per-core FC1 K-slice
N_CORES = 8

TRACE = False           # test.py flips this for profiled runs
_CACHE = {}

KXS = (-1, 0, 1)


def _build_nc():
    nc = bacc.Bacc("TRN2", target_bir_lowering=False, debug=False,
                   num_devices=N_CORES)

    # ---- I/O -------------------------------------------------------------
    xp_d = nc.dram_tensor("xp", [T, C, PH, PW], F16, kind="ExternalInput")
    w0_d = nc.dram_tensor("w0", [105, 3 * G4], F16, kind="ExternalInput")
    w1h0_d = nc.dram_tensor("w1h0", [96, 3 * G4], F16, kind="ExternalInput")
    w1h1_d = nc.dram_tensor("w1h1", [96, 3 * G4], F16, kind="ExternalInput")
    wd_d = nc.dram_tensor("wd", [105, 3 * G4], F16, kind="ExternalInput")
    b0_d = nc.dram_tensor("b0", [G4, 1], F32, kind="ExternalInput")
    b1_d = nc.dram_tensor("b1", [G4, 1], F32, kind="ExternalInput")
    bd_d = nc.dram_tensor("bd", [G4, 1], F32, kind="ExternalInput")
    fw_d = nc.dram_tensor("fw", [128, 128 * 256], F16, kind="ExternalInput")
    fb_d = nc.dram_tensor("fb", [128, 2], F32, kind="ExternalInput")
    w2_d = nc.dram_tensor("w2", [128, 2 * 97], F16, kind="ExternalInput")
    b2_d = nc.dram_tensor("b2", [97, 1], F32, kind="ExternalInput")
    out_d = nc.dram_tensor("out", [97, 1], F32, kind="ExternalOutput")

    with tile.TileContext(nc) as tc:
        with (
            tc.tile_pool(name="state", bufs=1) as state,
            tc.tile_pool(name="const", bufs=1) as const,
            tc.tile_pool(name="sgate", bufs=3) as sgate,
            tc.tile_pool(name="scr", bufs=3) as scr,
            tc.tile_pool(name="psum", bufs=4, space="PSUM") as psum,
            tc.tile_pool(name="dram", bufs=1, space="DRAM") as dram,
        ):
            # ---- persistent SBUF state ----------------------------------
            A = state.tile([105, PH, PW], F16)
            H1t = state.tile([96, PH, PW], F16)
            # ping-pong cell state, lane 0 = L0, lane 1 = L1/decoder
            cst = [state.tile([64, 2, NPIX], F16, name="cst_a"),
                   state.tile([64, 2, NPIX], F16, name="cst_b")]
            hdc = state.tile([HID, NPIX], F16)    # decoder h (feat)

            # ---- constants ----------------------------------------------
            w0 = const.tile([105, 3 * G4], F16)
            w1h0 = const.tile([96, 3 * G4], F16)
            w1h1 = const.tile([96, 3 * G4], F16)
            wd = const.tile([105, 3 * G4], F16)
            b0 = const.tile([G4, 1], F32)
            b1 = const.tile([G4, 1], F32)
            bd = const.tile([G4, 1], F32)
            fw = const.tile([128, 128 * 256], F16)
            fb = const.tile([128, 2], F32)
            w2 = const.tile([128, 2 * 97], F16)
            b2 = const.tile([97, 1], F32)
            ft = const.tile([128, 8, 128], F16)   # A2A result, FC1 lhsT

            # zero-init on DVE; first matmuls only need A + w0 + b0 + x(0)
            nc.vector.memset(A[:], 0.0)
            nc.sync.dma_start(out=w0[:], in_=w0_d[:])
            nc.sync.dma_start(out=b0[:], in_=b0_d[:])
            nc.vector.memset(H1t[:], 0.0)
            nc.vector.memset(cst[0][:], 0.0)
            nc.vector.memset(cst[1][:], 0.0)

            # ---- DRAM bounce buffers for collectives --------------------
            a2a_in = dram.tile([HID, NPIX], F16)
            a2a_out = dram.tile([8, 128, 128], F16)
            z1part = dram.tile([8, 256], F32)
            z1red = dram.tile([256], F32)

            def xload(t):
                # x_t into A's 3 ky-groups (grp_ky[d] = x[d+ky])
                nc.sync.dma_start(out=A[99:102, :, :], in_=xp_d[t])
                nc.sync.dma_start(out=A[96:99, 1:PH, :],
                                  in_=xp_d[t, :, 0:PH - 1, :])
                nc.sync.dma_start(out=A[102:105, 0:PH - 1, :],
                                  in_=xp_d[t, :, 1:PH, :])

            def conv_step(srcs, bias, R, Wt, li, hdst,
                          defer=False, guest=None, immediate=False,
                          dve_add=False, post_late=None):
                """One ConvLSTM cell step.  srcs: list of (buf, K, wt).
                R/Wt: ping-pong [64, 2, NPIX] cst tiles; li: lane (0=L0).

                Per quarter rt: matmul passes into a [128,1024] PSUM tile,
                then sigmoid + the c-chain: ts (tg into R), pair-mul
                ([i*tg|f*c] into W), cross-partition add W[32:64]+=W[0:32]
                (SWDGE accumulate, or DVE copy+add when dve_add).

                Tail policy: defer=True computes tanh inline but returns
                (g_a, g_b) closures -- g_a(q) = h-mul + ky+1 copy, g_b(q) =
                ky-1 copy -- for the next conv to host; guest=(g_a, g_b)
                emits them after this conv's matmuls q / q+1; immediate=True
                (decoder) emits the full tail right after each quarter.
                """
                npass = len(srcs) * 3
                S = sgate.tile([G4, NPIX], F16, tag="S")
                thts = {}

                def tanh_q(q):
                    tht = scr.tile([96, 1024], F16, tag="tht")
                    nc.scalar.activation(
                        out=tht[64:96, :],
                        in_=Wt[32:64, li, q * 1024:(q + 1) * 1024],
                        func=AF.Tanh)
                    thts[q] = tht

                def hmul_kyp1(q):
                    if q not in thts:
                        tanh_q(q)
                    s_ = slice(q * 1024, (q + 1) * 1024)
                    r0 = 16 * q
                    th = thts[q][64:96, :]
                    if hdst is hdc:
                        nc.vector.tensor_mul(hdc[:, s_], S[64:96, s_], th)
                    else:
                        nc.vector.tensor_mul(
                            hdst[32:64, r0 + 1:r0 + 17, 2:66],
                            S[64:96, s_], th)
                        nc.gpsimd.dma_start(
                            out=hdst[64:96, r0:r0 + 16, :],
                            in_=hdst[32:64, r0 + 1:r0 + 17, :])
                    if post_late is not None:
                        post_late(q)

                def kym1(q):
                    if hdst is hdc:
                        return
                    r0 = 16 * q
                    nc.sync.dma_start(
                        out=hdst[0:32, r0 + 2:r0 + 18, :],
                        in_=hdst[32:64, r0 + 1:r0 + 17, :])

                for rt in range(4):
                    s_ = slice(rt * 1024, (rt + 1) * 1024)
                    csl = slice(rt * 1024, (rt + 1) * 1024)
                    pz = psum.tile([G4, 1024], F32, tag="z", name=f"pz{rt}")
                    ip = 0
                    for buf, K, wt in srcs:
                        for kxi, kx in enumerate(KXS):
                            lhs = wt[:, kxi * G4:(kxi + 1) * G4]
                            for hh in range(2):
                                r0 = 16 * rt + 8 * hh
                                rhs = buf[0:K, r0 + 1:r0 + 9, 2 + kx:66 + kx]
                                nc.tensor.matmul(
                                    pz[:, 512 * hh:512 * hh + 512],
                                    lhs, rhs, start=(ip == 0),
                                    stop=(ip == npass - 1))
                            ip += 1
                    if guest is not None:
                        guest[0](rt)               # prev conv h-mul + ky+1
                        if rt >= 1:
                            guest[1](rt - 1)       # prev conv ky-1 copy
                    if not defer and not immediate and rt >= 1:
                        q = rt - 1
                        tanh_q(q)
                        hmul_kyp1(q)
                        kym1(q)
                    nc.scalar.activation(out=S[:, s_], in_=pz[:],
                                         func=AF.Sigmoid,
                                         bias=bias[:, 0:1], scale=1.0)
                    # tg = 2*sigmoid(2g) - 1 into R's (dead) lower half
                    nc.vector.tensor_scalar(
                        out=R[0:32, li, csl], in0=S[96:128, s_],
                        scalar1=2.0, scalar2=-1.0, op0=OP.mult, op1=OP.add)
                    # [i*tg | f*c]
                    nc.vector.tensor_mul(Wt[0:64, li, csl], S[0:64, s_],
                                         R[0:64, li, csl])
                    # c_new = i*tg + f*c (cross-partition add).  Quarter 3
                    # always uses the DVE path: its tanh can't hide under
                    # later quarters, and a ~3us SWDGE flight there blocks
                    # the ACT FIFO into the next conv's sigmoids.
                    if dve_add or rt == 3:
                        u1c = scr.tile([32, 1024], F16, tag="u1c")
                        nc.vector.tensor_copy(u1c[:], Wt[32:64, li, csl])
                        nc.vector.tensor_add(Wt[32:64, li, csl],
                                             Wt[0:32, li, csl], u1c[:])
                    else:
                        nc.gpsimd.dma_start(out=Wt[32:64, li, csl],
                                            in_=Wt[0:32, li, csl],
                                            accum_op=OP.add)
                    if defer and rt >= 1:
                        tanh_q(rt - 1)             # inline; placement rides
                    if immediate:
                        tanh_q(rt)
                        hmul_kyp1(rt)
                        kym1(rt)
                if defer:
                    tanh_q(3)                      # inline, DVE-add gated
                if guest is not None:
                    guest[1](3)
                if not defer and not immediate:
                    tanh_q(3)
                    hmul_kyp1(3)
                    kym1(3)
                return (hmul_kyp1, kym1)

            # ================= recurrent steps ===========================
            # Ping-pong parity: L0(t): R=cst[t%2], W=cst[(t+1)%2];
            # L1(s): R=cst[(s+1)%2], W=cst[s%2].
            xload(0)
            # remaining consts; behind w0/b0/x(0) on sync so the first
            # matmuls start ~2.5us in
            for dst, src in ((w1h0, w1h0_d), (w1h1, w1h1_d), (b1, b1_d),
                             (wd, wd_d), (bd, bd_d), (fb, fb_d),
                             (w2, w2_d), (b2, b2_d)):
                nc.sync.dma_start(out=dst[:], in_=src[:])
            pend = None
            for t in range(T):
                pend = conv_step([(A, 105, w0)], b0,
                                 cst[t % 2], cst[(t + 1) % 2], 0, A,
                                 defer=(t > 0))
                if t + 1 < T:
                    xload(t + 1)
                if t > 0:
                    s = t - 1
                    conv_step([(A, 96, w1h0), (H1t, 96, w1h1)], b1,
                              cst[(s + 1) % 2], cst[s % 2], 1, H1t,
                              guest=pend)
                if 1 <= t < 1 + 16:
                    # trickle in the 8.4MB fc1 weight (tail-only)
                    i = t - 1
                    nc.gpsimd.dma_start(out=fw[:, i * 2048:(i + 1) * 2048],
                                        in_=fw_d[:, i * 2048:(i + 1) * 2048])
            # final L1 step (s = T-1): hT fans out into A's h slots; its
            # tail must be inline (the decoder READS those writes).
            s = T - 1
            conv_step([(A, 96, w1h0), (H1t, 96, w1h1)], b1,
                      cst[(s + 1) % 2], cst[s % 2], 1, A, dve_add=True)

            # ================= decoder step ==============================
            def feed(q):
                # stream each decoder quarter into the A2A input
                sl = slice(q * 1024, (q + 1) * 1024)
                nc.sync.dma_start(out=a2a_in[:, sl], in_=hdc[:, sl])

            conv_step([(A, 105, wd)], bd,
                      cst[(T + 1) % 2], cst[T % 2], 1, hdc,
                      immediate=True, dve_add=True, post_late=feed)

            # ================= FC head ===================================
            nc.gpsimd.collective_compute(
                "AllToAll", OP.bypass,
                replica_groups=[list(range(N_CORES))],
                ins=[a2a_in[:].opt()], outs=[a2a_out[:].opt()])
            # transposed load with K-index q = p*128 + k2
            nc.sync.dma_start(
                out=ft[:],
                in_=a2a_out[:].rearrange("m p k -> p m k"))

            psz = psum.tile([8, 256], F32, tag="z")
            for k2 in range(128):
                nc.tensor.matmul(psz[:], ft[:, :, k2],
                                 fw[:, k2 * 256:(k2 + 1) * 256],
                                 start=(k2 == 0), stop=(k2 == 127))
            z1s = scr.tile([8, 256], F32, tag="z1")
            nc.vector.tensor_copy(z1s[:], psz[:])
            nc.sync.dma_start(out=z1part[:], in_=z1s[:])
            nc.gpsimd.collective_compute(
                "ReduceScatter", OP.add,
                replica_groups=[list(range(N_CORES))],
                ins=[z1part[:].opt()], outs=[z1red[:].opt()])

            zr = scr.tile([128, 2], F32, tag="zr")
            nc.sync.dma_start(out=zr[:],
                              in_=z1red[:].rearrange("(j p) -> p j", p=128))
            zrb = scr.tile([128, 2], F32, tag="zrb")
            nc.vector.tensor_add(zrb[:], zr[:], fb[:])
            h256 = scr.tile([128, 2], F16, tag="h256")
            nc.vector.tensor_scalar_max(h256[:], zrb[:], 0.0)

            ps2 = psum.tile([97, 1], F32, tag="z")
            for j in range(2):
                nc.tensor.matmul(ps2[:], w2[:, j * 97:(j + 1) * 97],
                                 h256[:, j:j + 1],
                                 start=(j == 0), stop=(j == 1))
            outs = scr.tile([97, 1], F32, tag="outs")
            nc.vector.tensor_add(outs[:], ps2[:], b2[:])
            nc.sync.dma_start(out=out_d[:], in_=outs[:])

    nc.compile()
    return nc


def _prep_inputs(x, Wenc0, benc0, Wenc1, benc1, Wdec, bdec,
                 fc1_w, fc1_b, fc2_w, fc2_b):
    """Host-side: pad/reorder/cast everything into device layouts."""
    f16 = np.float16

    def conv_w_l0(Wk):
        # Wk [128, 35, 3, 3]; ref channel order [x(3), h(32)]
        Wk = np.asarray(Wk, np.float32).copy()
        Wk[96:128] *= 2.0
        out = np.zeros((105, 3 * G4), np.float32)
        for kxi in range(3):
            for dy in range(3):
                out[32 * dy:32 * dy + 32, kxi * G4:(kxi + 1) * G4] = \
                    Wk[:, 3:, dy, kxi].T
                out[96 + 3 * dy:99 + 3 * dy, kxi * G4:(kxi + 1) * G4] = \
                    Wk[:, :3, dy, kxi].T
        return out.astype(f16)

    def conv_w_l1(Wk):
        # Wk [128, 64, 3, 3]; ref channel order [h0(32), h1(32)]
        Wk = np.asarray(Wk, np.float32).copy()
        Wk[96:128] *= 2.0
        wh0 = np.zeros((96, 3 * G4), np.float32)
        wh1 = np.zeros((96, 3 * G4), np.float32)
        for kxi in range(3):
            for dy in range(3):
                wh0[32 * dy:32 * dy + 32, kxi * G4:(kxi + 1) * G4] = \
                    Wk[:, 0:32, dy, kxi].T
                wh1[32 * dy:32 * dy + 32, kxi * G4:(kxi + 1) * G4] = \
                    Wk[:, 32:64, dy, kxi].T
        return wh0.astype(f16), wh1.astype(f16)

    def bias_v(b):
        b = np.asarray(b, np.float32).copy()
        b[96:128] *= 2.0
        return b.reshape(G4, 1)

    w0_full = conv_w_l0(Wenc0)
    wd_full = conv_w_l0(Wdec)
    w1h0, w1h1 = conv_w_l1(Wenc1)

    xpad = np.zeros((B, T, C, PH, PW), f16)
    xpad[:, :, :, 1:65, 2:66] = np.asarray(x, np.float32)

    fc1_w = np.asarray(fc1_w, np.float32)
    fb = np.asarray(fc1_b, np.float32).reshape(2, 128).T.copy()  # [128, 2]
    w2 = np.asarray(fc2_w, np.float32).T.reshape(2, 128, 97)
    w2 = np.ascontiguousarray(w2.transpose(1, 0, 2)).reshape(128, 2 * 97)
    b2 = np.asarray(fc2_b, np.float32).reshape(97, 1)

    in_maps = []
    for k in range(N_CORES):
        w1k = fc1_w[:, k * KSL:(k + 1) * KSL].T            # [16384, 256]
        # K-index q = p*128 + k2  ->  fw[p, k2*256+n] = w1k[p*128 + k2, n]
        fwk = w1k.reshape(128, 128 * 256)
        in_maps.append({
            "xp": np.ascontiguousarray(xpad[k]),
            "w0": w0_full, "w1h0": w1h0, "w1h1": w1h1, "wd": wd_full,
            "b0": bias_v(benc0), "b1": bias_v(benc1), "bd": bias_v(bdec),
            "fw": fwk.astype(f16), "fb": fb,
            "w2": w2.astype(f16), "b2": b2,
        })
    return in_maps


def kernel(**inputs):
    if "nc" not in _CACHE:
        _CACHE["nc"] = _build_nc()
    nc = _CACHE["nc"]
    in_maps = _prep_inputs(**inputs)
    res = run_bass_kernel_spmd(nc, in_maps, core_ids=list(range(N_CORES)),
                               trace=TRACE)
    _CACHE["last_result"] = res
    out = np.stack([res.results[k]["out"][:, 0] for k in range(N_CORES)])
    return out.astype(np.float32)


# revision 7
# speedup vs baseline: 1.2362x; 1.0167x over previous
"""Trainium2 Bass kernel for nn_BaltNet (2-layer ConvLSTM + decoder + MLP head).

Sharding: data-parallel over batch B=8 (one sample per NeuronCore) for the
recurrent conv part; FC1's [131072, 256] contraction is K-sharded 8 ways
(AllToAll of the decoder features, per-core partial matmul, ReduceScatter).

Layout (v4): L1 shares A's h0 ky-groups, so h0 needs no separate Ba/Bb
placements (v1 kept 7 shifted-copy DMAs per quarter; the sync queue was
~10us backed up at every step boundary and re-throttled the PE).

  A  [105, 66, 68]: h0 ky=-1 @0-31, ky=0 @32-63, ky=+1 @64-95;
                    x  ky=-1 @96-98, ky=0 @99-101, ky=+1 @102-104
  H1 [ 96, 66, 68]: h1 ky=-1 @0-31, ky=0 @32-63, ky=+1 @64-95

  L0 / decoder: 3 kx passes of K=105 over A.
  L1:           3 kx passes of K=96 over A[0:96] + 3 of K=96 over H1.

All ky groups are 32-aligned: the h producer (DVE mul) writes the ky=0 slot
directly and only 2 shifted copies per quarter remain (sync + gpsimd DMA).

Pointwise: gates z = [i f o g] on 128 partitions; g-gate weights/bias
pre-scaled x2 so tanh(g) = 2*sigmoid(2g) - 1.  Cell state uses ping-pong
[64, 2, 4096] tiles (lane 0 = L0, lane 1 = L1/decoder) R/W per step:
R = [tg | c_old], the pair-mul writes W[0:64] = [i*tg | f*c], and a SWDGE
accumulate-DMA does the cross-partition add W[32:64] += W[0:32] = c_new
(saves 2 DVE ops/quarter vs v1's copy+add; DVE was 78%-busy).  The decoder
and final-L1 steps use a DVE copy+add instead: at the tail nothing hides
the ~3us SWDGE flight, and the shorter chain triggers the AllToAll sooner.

Scheduling: L1 one step behind L0, emitted after it, so the PE alternates
L0(t) / L1(t-1) bursts.  L0(t) computes its tanh inline (gated only by its
own c-update), and ONLY its h-placements (h-mul + 2 shifted copies) are
deferred to ride in L1(t-1): quarter q's h-mul + ky+1 copy after L1's
matmuls q (their A-rows WAR-clear there), the ky-1 copy after matmuls q+1
(one boundary row).  Placements finish under L1's burst, so L0(t+1) starts
stall-free and the PE's HAM activity throttle stays released (in v1-v3 a
per-step PE gap re-throttled the clock to 1.2GHz for ~half of every step).

Tail: single AllToAll + single ReduceScatter (two phased A2As measured 5x
slower than one -- per-collective overhead dominates at these sizes).
"""

import os
import sys

for _p in ("/opt/trn_rl_repo",):
    if _p not in sys.path and os.path.isdir(_p):
        sys.path.insert(0, _p)

import numpy as np

import concourse.bass as bass
import concourse.mybir as mybir
import concourse.tile as tile
from concourse import bacc
from concourse.bass_utils import run_bass_kernel_spmd

F16 = mybir.dt.float16
F32 = mybir.dt.float32
AF = mybir.ActivationFunctionType
OP = mybir.AluOpType

B, T, C, HID, H, W = 8, 24, 3, 32, 64, 64
G4 = 4 * HID            # 128 gate channels
PH, PW = H + 2, W + 4   # padded spatial: rows 0..65, interior cols 2..65
NPIX = H * W            # 4096
KSL = HID * NPIX // 8   # 16384 per-core FC1 K-slice
N_CORES = 8

TRACE = False           # test.py flips this for profiled runs
_CACHE = {}

KXS = (-1, 0, 1)


def _build_nc():
    nc = bacc.Bacc("TRN2", target_bir_lowering=False, debug=False,
                   num_devices=N_CORES)

    # ---- I/O -------------------------------------------------------------
    xp_d = nc.dram_tensor("xp", [T, C, PH, PW], F16, kind="ExternalInput")
    w0_d = nc.dram_tensor("w0", [105, 3 * G4], F16, kind="ExternalInput")
    w1h0_d = nc.dram_tensor("w1h0", [96, 3 * G4], F16, kind="ExternalInput")
    w1h1_d = nc.dram_tensor("w1h1", [96, 3 * G4], F16, kind="ExternalInput")
    wd_d = nc.dram_tensor("wd", [105, 3 * G4], F16, kind="ExternalInput")
    b0_d = nc.dram_tensor("b0", [G4, 1], F32, kind="ExternalInput")
    b1_d = nc.dram_tensor("b1", [G4, 1], F32, kind="ExternalInput")
    bd_d = nc.dram_tensor("bd", [G4, 1], F32, kind="ExternalInput")
    fw_d = nc.dram_tensor("fw", [128, 128 * 256], F16, kind="ExternalInput")
    fb_d = nc.dram_tensor("fb", [128, 2], F32, kind="ExternalInput")
    w2_d = nc.dram_tensor("w2", [128, 2 * 97], F16, kind="ExternalInput")
    b2_d = nc.dram_tensor("b2", [97, 1], F32, kind="ExternalInput")
    out_d = nc.dram_tensor("out", [97, 1], F32, kind="ExternalOutput")

    with tile.TileContext(nc) as tc:
        with (
            tc.tile_pool(name="state", bufs=1) as state,
            tc.tile_pool(name="const", bufs=1) as const,
            tc.tile_pool(name="sgate", bufs=4) as sgate,
            tc.tile_pool(name="scr", bufs=4) as scr,
            tc.tile_pool(name="psum", bufs=4, space="PSUM") as psum,
            tc.tile_pool(name="dram", bufs=1, space="DRAM") as dram,
        ):
            # ---- persistent SBUF state ----------------------------------
            A = state.tile([105, PH, PW], F16)
            H1t = state.tile([96, PH, PW], F16)
            # ping-pong cell state, per layer (L1 tiles also serve the
            # decoder); separate tiles so the two layers' c-chains never
            # alias in the dependency tracker
            cst0 = [state.tile([64, NPIX], F16, name="cst0_a"),
                    state.tile([64, NPIX], F16, name="cst0_b")]
            cst1 = [state.tile([64, NPIX], F16, name="cst1_a"),
                    state.tile([64, NPIX], F16, name="cst1_b")]
            hdc = state.tile([HID, NPIX], F16)    # decoder h (feat)

            # ---- constants ----------------------------------------------
            w0 = const.tile([105, 3 * G4], F16)
            w1h0 = const.tile([96, 3 * G4], F16)
            w1h1 = const.tile([96, 3 * G4], F16)
            wd = const.tile([105, 3 * G4], F16)
            b0 = const.tile([G4, 1], F32)
            b1 = const.tile([G4, 1], F32)
            bd = const.tile([G4, 1], F32)
            fw = const.tile([128, 128 * 256], F16)
            fb = const.tile([128, 2], F32)
            w2 = const.tile([128, 2 * 97], F16)
            b2 = const.tile([97, 1], F32)
            ft = const.tile([128, 8, 128], F16)   # A2A result, FC1 lhsT

            # zero-init on DVE; first matmuls only need A + w0 + b0 + x(0)
            nc.vector.memset(A[:], 0.0)
            nc.sync.dma_start(out=w0[:], in_=w0_d[:])
            nc.sync.dma_start(out=b0[:], in_=b0_d[:])
            nc.vector.memset(H1t[:], 0.0)
            nc.vector.memset(cst0[0][:], 0.0)
            nc.vector.memset(cst1[0][:], 0.0)
            nc.vector.memset(cst1[1][:], 0.0)

            # ---- DRAM bounce buffers for collectives --------------------
            a2a_in = dram.tile([HID, NPIX], F16)
            a2a_out = dram.tile([8, 128, 128], F16)
            z1part = dram.tile([8, 256], F32)
            z1red = dram.tile([256], F32)

            def xload(t):
                # x_t into A's 3 ky-groups (grp_ky[d] = x[d+ky])
                nc.sync.dma_start(out=A[99:102, :, :], in_=xp_d[t])
                nc.sync.dma_start(out=A[96:99, 1:PH, :],
                                  in_=xp_d[t, :, 0:PH - 1, :])
                nc.sync.dma_start(out=A[102:105, 0:PH - 1, :],
                                  in_=xp_d[t, :, 1:PH, :])

            def conv_step(srcs, bias, R, Wt, hdst,
                          defer=False, guest=None, immediate=False,
                          dve_add=False, post_late=None):
                """One ConvLSTM cell step.  srcs: list of (buf, K, wt).
                R/Wt: the layer's ping-pong [64, NPIX] cst tiles.

                Per quarter rt: matmul passes into a [128,1024] PSUM tile,
                then sigmoid + the c-chain: ts (tg into R), pair-mul
                ([i*tg|f*c] into W), cross-partition add W[32:64]+=W[0:32]
                (SWDGE accumulate, or DVE copy+add when dve_add).

                Tail policy: defer=True computes tanh inline but returns
                (g_a, g_b) closures -- g_a(q) = h-mul + ky+1 copy, g_b(q) =
                ky-1 copy -- for the next conv to host; guest=(g_a, g_b)
                emits them after this conv's matmuls q / q+1; immediate=True
                (decoder) emits the full tail right after each quarter.
                """
                npass = len(srcs) * 3
                S = sgate.tile([G4, NPIX], F16, tag="S")
                thts = {}

                def tanh_q(q):
                    tht = scr.tile([96, 1024], F16, tag="tht")
                    nc.scalar.activation(
                        out=tht[64:96, :],
                        in_=Wt[32:64, q * 1024:(q + 1) * 1024],
                        func=AF.Tanh)
                    thts[q] = tht

                def hmul_kyp1(q):
                    if q not in thts:
                        tanh_q(q)
                    s_ = slice(q * 1024, (q + 1) * 1024)
                    r0 = 16 * q
                    th = thts[q][64:96, :]
                    if hdst is hdc:
                        nc.vector.tensor_mul(hdc[:, s_], S[64:96, s_], th)
                    else:
                        nc.vector.tensor_mul(
                            hdst[32:64, r0 + 1:r0 + 17, 2:66],
                            S[64:96, s_], th)
                        nc.gpsimd.dma_start(
                            out=hdst[64:96, r0:r0 + 16, :],
                            in_=hdst[32:64, r0 + 1:r0 + 17, :])
                    if post_late is not None:
                        post_late(q)

                def kym1(q):
                    if hdst is hdc:
                        return
                    r0 = 16 * q
                    nc.sync.dma_start(
                        out=hdst[0:32, r0 + 2:r0 + 18, :],
                        in_=hdst[32:64, r0 + 1:r0 + 17, :])

                for rt in range(4):
                    s_ = slice(rt * 1024, (rt + 1) * 1024)
                    csl = slice(rt * 1024, (rt + 1) * 1024)
                    pz = psum.tile([G4, 1024], F32, tag="z", name=f"pz{rt}")
                    ip = 0
                    for buf, K, wt in srcs:
                        for kxi, kx in enumerate(KXS):
                            lhs = wt[:, kxi * G4:(kxi + 1) * G4]
                            for hh in range(2):
                                r0 = 16 * rt + 8 * hh
                                rhs = buf[0:K, r0 + 1:r0 + 9, 2 + kx:66 + kx]
                                nc.tensor.matmul(
                                    pz[:, 512 * hh:512 * hh + 512],
                                    lhs, rhs, start=(ip == 0),
                                    stop=(ip == npass - 1))
                            ip += 1
                    if guest is not None:
                        guest[0](rt)               # prev conv h-mul + ky+1
                        if rt >= 1:
                            guest[1](rt - 1)       # prev conv ky-1 copy
                    if not defer and not immediate and rt >= 1:
                        q = rt - 1
                        tanh_q(q)
                        hmul_kyp1(q)
                        kym1(q)
                    nc.scalar.activation(out=S[:, s_], in_=pz[:],
                                         func=AF.Sigmoid,
                                         bias=bias[:, 0:1], scale=1.0)
                    # tg = 2*sigmoid(2g) - 1 into R's (dead) lower half
                    nc.vector.tensor_scalar(
                        out=R[0:32, csl], in0=S[96:128, s_],
                        scalar1=2.0, scalar2=-1.0, op0=OP.mult, op1=OP.add)
                    # [i*tg | f*c]
                    nc.vector.tensor_mul(Wt[0:64, csl], S[0:64, s_],
                                         R[0:64, csl])
                    # c_new = i*tg + f*c (cross-partition add).  Quarter 3
                    # always uses the DVE path: its tanh can't hide under
                    # later quarters, and a ~3us SWDGE flight there blocks
                    # the ACT FIFO into the next conv's sigmoids.
                    if dve_add or rt == 3:
                        u1c = scr.tile([32, 1024], F16, tag="u1c")
                        nc.vector.tensor_copy(u1c[:], Wt[32:64, csl])
                        nc.vector.tensor_add(Wt[32:64, csl],
                                             Wt[0:32, csl], u1c[:])
                    else:
                        nc.gpsimd.dma_start(out=Wt[32:64, csl],
                                            in_=Wt[0:32, csl],
                                            accum_op=OP.add)
                    if defer and rt >= 1:
                        tanh_q(rt - 1)             # inline; placement rides
                    if immediate:
                        tanh_q(rt)
                        hmul_kyp1(rt)
                        kym1(rt)
                if defer:
                    tanh_q(3)                      # inline, DVE-add gated
                if guest is not None:
                    guest[1](3)
                if not defer and not immediate:
                    tanh_q(3)
                    hmul_kyp1(3)
                    kym1(3)
                return (hmul_kyp1, kym1)

            # ================= recurrent steps ===========================
            # Ping-pong parity: L0(t): R=cst[t%2], W=cst[(t+1)%2];
            # L1(s): R=cst[(s+1)%2], W=cst[s%2].
            xload(0)
            # remaining consts; behind w0/b0/x(0) on sync so the first
            # matmuls start ~2.5us in
            for dst, src in ((w1h0, w1h0_d), (w1h1, w1h1_d), (b1, b1_d),
                             (wd, wd_d), (bd, bd_d), (fb, fb_d),
                             (w2, w2_d), (b2, b2_d)):
                nc.sync.dma_start(out=dst[:], in_=src[:])
            pend = None
            for t in range(T):
                pend = conv_step([(A, 105, w0)], b0,
                                 cst0[t % 2], cst0[(t + 1) % 2], A,
                                 defer=(t > 0))
                if t + 1 < T:
                    xload(t + 1)
                if t > 0:
                    s = t - 1
                    conv_step([(A, 96, w1h0), (H1t, 96, w1h1)], b1,
                              cst1[(s + 1) % 2], cst1[s % 2], H1t,
                              guest=pend)
                if 1 <= t < 1 + 16:
                    # trickle in the 8.4MB fc1 weight (tail-only)
                    i = t - 1
                    nc.gpsimd.dma_start(out=fw[:, i * 2048:(i + 1) * 2048],
                                        in_=fw_d[:, i * 2048:(i + 1) * 2048])
            # final L1 step (s = T-1): hT fans out into A's h slots; its
            # tail must be inline (the decoder READS those writes).
            s = T - 1
            conv_step([(A, 96, w1h0), (H1t, 96, w1h1)], b1,
                      cst1[(s + 1) % 2], cst1[s % 2], A, dve_add=True)

            # ================= decoder step ==============================
            def feed(q):
                # stream each decoder quarter into the A2A input
                sl = slice(q * 1024, (q + 1) * 1024)
                nc.sync.dma_start(out=a2a_in[:, sl], in_=hdc[:, sl])

            conv_step([(A, 105, wd)], bd,
                      cst1[(T + 1) % 2], cst1[T % 2], hdc,
                      immediate=True, dve_add=True, post_late=feed)

            # ================= FC head ===================================
            nc.gpsimd.collective_compute(
                "AllToAll", OP.bypass,
                replica_groups=[list(range(N_CORES))],
                ins=[a2a_in[:].opt()], outs=[a2a_out[:].opt()])
            # transposed load with K-index q = p*128 + k2
            nc.sync.dma_start(
                out=ft[:],
                in_=a2a_out[:].rearrange("m p k -> p m k"))

            psz = psum.tile([8, 256], F32, tag="z")
            for k2 in range(128):
                nc.tensor.matmul(psz[:], ft[:, :, k2],
                                 fw[:, k2 * 256:(k2 + 1) * 256],
                                 start=(k2 == 0), stop=(k2 == 127))
            z1s = scr.tile([8, 256], F32, tag="z1")
            nc.vector.tensor_copy(z1s[:], psz[:])
            nc.sync.dma_start(out=z1part[:], in_=z1s[:])
            nc.gpsimd.collective_compute(
                "ReduceScatter", OP.add,
                replica_groups=[list(range(N_CORES))],
                ins=[z1part[:].opt()], outs=[z1red[:].opt()])

            zr = scr.tile([128, 2], F32, tag="zr")
            nc.sync.dma_start(out=zr[:],
                              in_=z1red[:].rearrange("(j p) -> p j", p=128))
            zrb = scr.tile([128, 2], F32, tag="zrb")
            nc.vector.tensor_add(zrb[:], zr[:], fb[:])
            h256 = scr.tile([128, 2], F16, tag="h256")
            nc.vector.tensor_scalar_max(h256[:], zrb[:], 0.0)

            ps2 = psum.tile([97, 1], F32, tag="z")
            for j in range(2):
                nc.tensor.matmul(ps2[:], w2[:, j * 97:(j + 1) * 97],
                                 h256[:, j:j + 1],
                                 start=(j == 0), stop=(j == 1))
            outs = scr.tile([97, 1], F32, tag="outs")
            nc.vector.tensor_add(outs[:], ps2[:], b2[:])
            nc.sync.dma_start(out=out_d[:], in_=outs[:])

    nc.compile()
    return nc


def _prep_inputs(x, Wenc0, benc0, Wenc1, benc1, Wdec, bdec,
                 fc1_w, fc1_b, fc2_w, fc2_b):
    """Host-side: pad/reorder/cast everything into device layouts."""
    f16 = np.float16

    def conv_w_l0(Wk):
        # Wk [128, 35, 3, 3]; ref channel order [x(3), h(32)]
        Wk = np.asarray(Wk, np.float32).copy()
        Wk[96:128] *= 2.0
        out = np.zeros((105, 3 * G4), np.float32)
        for kxi in range(3):
            for dy in range(3):
                out[32 * dy:32 * dy + 32, kxi * G4:(kxi + 1) * G4] = \
                    Wk[:, 3:, dy, kxi].T
                out[96 + 3 * dy:99 + 3 * dy, kxi * G4:(kxi + 1) * G4] = \
                    Wk[:, :3, dy, kxi].T
        return out.astype(f16)

    def conv_w_l1(Wk):
        # Wk [128, 64, 3, 3]; ref channel order [h0(32), h1(32)]
        Wk = np.asarray(Wk, np.float32).copy()
        Wk[96:128] *= 2.0
        wh0 = np.zeros((96, 3 * G4), np.float32)
        wh1 = np.zeros((96, 3 * G4), np.float32)
        for kxi in range(3):
            for dy in range(3):
                wh0[32 * dy:32 * dy + 32, kxi * G4:(kxi + 1) * G4] = \
                    Wk[:, 0:32, dy, kxi].T
                wh1[32 * dy:32 * dy + 32, kxi * G4:(kxi + 1) * G4] = \
                    Wk[:, 32:64, dy, kxi].T
        return wh0.astype(f16), wh1.astype(f16)

    def bias_v(b):
        b = np.asarray(b, np.float32).copy()
        b[96:128] *= 2.0
        return b.reshape(G4, 1)

    w0_full = conv_w_l0(Wenc0)
    wd_full = conv_w_l0(Wdec)
    w1h0, w1h1 = conv_w_l1(Wenc1)

    xpad = np.zeros((B, T, C, PH, PW), f16)
    xpad[:, :, :, 1:65, 2:66] = np.asarray(x, np.float32)

    fc1_w = np.asarray(fc1_w, np.float32)
    fb = np.asarray(fc1_b, np.float32).reshape(2, 128).T.copy()  # [128, 2]
    w2 = np.asarray(fc2_w, np.float32).T.reshape(2, 128, 97)
    w2 = np.ascontiguousarray(w2.transpose(1, 0, 2)).reshape(128, 2 * 97)
    b2 = np.asarray(fc2_b, np.float32).reshape(97, 1)

    in_maps = []
    for k in range(N_CORES):
        w1k = fc1_w[:, k * KSL:(k + 1) * KSL].T            # [16384, 256]
        # K-index q = p*128 + k2  ->  fw[p, k2*256+n] = w1k[p*128 + k2, n]
        fwk = w1k.reshape(128, 128 * 256)
        in_maps.append({
            "xp": np.ascontiguousarray(xpad[k]),
            "w0": w0_full, "w1h0": w1h0, "w1h1": w1h1, "wd": wd_full,
            "b0": bias_v(benc0), "b1": bias_v(benc1), "bd": bias_v(bdec),
            "fw": fwk.astype(f16), "fb": fb,
            "w2": w2.astype(f16), "b2": b2,
        })
    return in_maps


def kernel(**inputs):
    if "nc" not in _CACHE:
        _CACHE["nc"] = _build_nc()
    nc = _CACHE["nc"]
    in_maps = _prep_inputs(**inputs)
    res = run_bass_kernel_spmd(nc, in_maps, core_ids=list(range(N_CORES)),
                               trace=TRACE)
    _CACHE["last_result"] = res
    out = np.stack([res.results[k]["out"][:, 0] for k in range(N_CORES)])
    return out.astype(np.float32)
